# revision 1
# baseline (speedup 1.0000x reference)
"""Trainium2 Bass kernel for nn_DECSeq3 (DynamicEdgeConv over streamlines).

Self-contained: hardcodes shapes from the problem spec.
  pos [131072, 3] f32, edge_index [2, 245760] int64, plus MLP weights.
  Output [8192, 2] f32.

Strategy: data-parallel over the 8192 streamlines across 8 NeuronCores
(1024 streamlines/core).  All BatchNorm affines that commute with
relu/max are folded into downstream weights on the host.  Each core:
  - stage1 pointwise convs (feature-major matmuls, fused bias via ones-row)
  - per-streamline kNN: block distance matmuls (psi/phi trick), top-5 via
    max8/max_index, neighbor-feature gather via indirect DMA from an HBM
    staging table, max aggregation on DVE
  - edge MLP via A/B decomposition: relu(A[p] + max_k B[nbr_k(p)])
  - l1 matmul + max-pool over points (strided grouped reduce), m1/m2/m3.
"""

import os
import sys

if "/opt/trn_rl_repo" not in sys.path:
    sys.path.insert(0, "/opt/trn_rl_repo")

import numpy as np

# ---------------- problem constants ----------------
B_FULL = 8192
L = 16
D = 3
K = 5
NCLS = 2
P = L - 1          # 15 real points per streamline
PP = 16            # padded points
EPS = 1e-5

NCORES = 8
BC = 1024          # streamlines per core
NODES = BC * PP    # 16384 padded nodes per core
NTILES = 16
TNODES = NODES // NTILES      # 2048 nodes per tile
TSTRL = BC // NTILES          # 128 streamlines per tile
NBLK = TNODES // 128          # 16 blocks of 128 nodes per tile
CHUNK = 512
BIG_NEG = -1.0e30

_CACHE = {}


# ---------------- device program ----------------
def _build_program():
    import concourse.bacc as bacc
    import concourse.bass as bass
    import concourse.mybir as mybir
    from concourse.tile import TileContext
    from concourse.masks import make_identity

    dt = mybir.dt
    f32 = dt.float32
    f32r = dt.float32r
    u32 = dt.uint32
    AF = mybir.ActivationFunctionType
    OP = mybir.AluOpType
    AX = mybir.AxisListType

    nc = bacc.Bacc("TRN2", target_bir_lowering=False)

    # ---- DRAM I/O ----
    xefw = nc.dram_tensor("xefw", [7, NODES], f32, kind="ExternalInput")
    xebw = nc.dram_tensor("xebw", [7, NODES], f32, kind="ExternalInput")
    s1wf = nc.dram_tensor("s1wf", [7, 64], f32, kind="ExternalInput")
    s1wb = nc.dram_tensor("s1wb", [7, 64], f32, kind="ExternalInput")
    s1g = nc.dram_tensor("s1g", [64, 1], f32, kind="ExternalInput")
    s1b = nc.dram_tensor("s1b", [64, 1], f32, kind="ExternalInput")
    wa = nc.dram_tensor("wa", [65, 128], f32r, kind="ExternalInput")
    wdt = nc.dram_tensor("wdt", [64, 128], f32, kind="ExternalInput")
    wl1x1 = nc.dram_tensor("wl1x1", [65, 1024], f32r, kind="ExternalInput")
    wl1x2 = nc.dram_tensor("wl1x2", [128, 1024], f32r, kind="ExternalInput")
    # m-layer weights pre-arranged on host to [128, kchunks*M]
    wm1 = nc.dram_tensor("wm1", [128, 8 * 512], f32r, kind="ExternalInput")
    bm1 = nc.dram_tensor("bm1", [1, 512], f32r, kind="ExternalInput")
    wm2 = nc.dram_tensor("wm2", [128, 4 * 256], f32r, kind="ExternalInput")
    bm2 = nc.dram_tensor("bm2", [1, 256], f32r, kind="ExternalInput")
    wm3 = nc.dram_tensor("wm3", [128, 2 * 2], f32r, kind="ExternalInput")
    bm3 = nc.dram_tensor("bm3", [1, 2], f32r, kind="ExternalInput")
    basep = nc.dram_tensor("basep", [128, NODES // NTILES // 128], u32, kind="ExternalInput")
    onesr = nc.dram_tensor("onesr", [1, BC], f32r, kind="ExternalInput")
    out_t = nc.dram_tensor("out", [2, BC], f32, kind="ExternalOutput")

    with TileContext(nc) as tc:
        with tc.tile_pool(name="const", bufs=1) as cpool, \
             tc.tile_pool(name="wpool", bufs=1) as wpool, \
             tc.tile_pool(name="pooled", bufs=1) as plpool, \
             tc.tile_pool(name="head", bufs=1) as headp, \
             tc.tile_pool(name="dram", bufs=1, space="DRAM") as dram:

            ident = cpool.tile([128, 128], f32)
            make_identity(nc, ident[:])
            ones_row = cpool.tile([1, BC], f32r)
            nc.sync.dma_start(out=ones_row[:], in_=onesr[:])

            t_s1w = wpool.tile([39, 64], f32)
            t_s1wf = t_s1w[0:7, :]
            t_s1wb = t_s1w[32:39, :]
            nc.sync.dma_start(out=t_s1wf, in_=s1wf[:])
            nc.sync.dma_start(out=t_s1wb, in_=s1wb[:])
            t_s1g = wpool.tile([64, 1], f32)
            nc.sync.dma_start(out=t_s1g[:], in_=s1g[:])
            t_s1b = wpool.tile([64, 1], f32)
            nc.sync.dma_start(out=t_s1b[:], in_=s1b[:])
            t_wa = wpool.tile([65, 128], f32r)
            nc.sync.dma_start(out=t_wa[:], in_=wa[:])
            t_wdt = wpool.tile([64, 128], f32)
            nc.sync.dma_start(out=t_wdt[:], in_=wdt[:])
            t_wl1x1 = wpool.tile([65, 1024], f32r)
            nc.sync.dma_start(out=t_wl1x1[:], in_=wl1x1[:])
            t_wl1x2 = wpool.tile([128, 1024], f32r)
            nc.sync.dma_start(out=t_wl1x2[:], in_=wl1x2[:])
            t_wm1 = wpool.tile([128, 8 * 512], f32r)
            nc.sync.dma_start(out=t_wm1[:], in_=wm1[:])
            t_bm1 = wpool.tile([1, 512], f32r)
            nc.sync.dma_start(out=t_bm1[:], in_=bm1[:])
            t_wm2 = wpool.tile([128, 4 * 256], f32r)
            nc.sync.dma_start(out=t_wm2[:], in_=wm2[:])
            t_bm2 = wpool.tile([1, 256], f32r)
            nc.sync.dma_start(out=t_bm2[:], in_=bm2[:])
            t_wm3 = wpool.tile([128, 4], f32r)
            nc.sync.dma_start(out=t_wm3[:], in_=wm3[:])
            t_bm3 = wpool.tile([1, 2], f32r)
            nc.sync.dma_start(out=t_bm3[:], in_=bm3[:])
            t_basep = wpool.tile([128, NODES // NTILES // 128], u32)
            nc.sync.dma_start(out=t_basep[:], in_=basep[:])

            b_hbms = [dram.tile([TNODES, 128], f32, name=f"b_hbm{i}",
                                tag=f"b_hbm{i}") for i in range(NTILES)]

            # pooled pre-activations, one [128, BC] buffer per 128-ch chunk
            pooled = [plpool.tile([128, BC], f32r, name=f"pooled{m}",
                                  tag=f"pooled{m}") for m in range(8)]

            with tc.tile_pool(name="io", bufs=2) as iop, \
                 tc.tile_pool(name="s1st", bufs=2) as s1st, \
                 tc.tile_pool(name="xt", bufs=2) as xtp, \
                 tc.tile_pool(name="knn", bufs=2) as knnp, \
                 tc.tile_pool(name="gat", bufs=2) as gatp, \
                 tc.tile_pool(name="bst", bufs=2) as bstp, \
                 tc.tile_pool(name="ps_mix", bufs=2, space="PSUM") as ps_mix, \
                 tc.tile_pool(name="ps_big", bufs=2, space="PSUM") as ps_big:

                ABLS = set(os.environ.get("KABL", "").split(","))
                ST = {}

                def ph1(t):
                    c0 = t * TNODES
                    # x1g: rows 0-63 = x1 (f32), row 64 unused, rows 64-127 = -1
                    x1g = xtp.tile([128, TNODES], f32, tag="x1g", name=f"x1g{t}")
                    x1t = xtp.tile([65, TNODES], f32r, tag="x1t", name=f"x1t{t}")
                    x1r2 = xtp.tile([128, TNODES], f32, tag="x1r2", name=f"x1r2{t}")
                    x2t = xtp.tile([128, TNODES], f32r, tag="x2t", name=f"x2t{t}")
                    ST[t] = dict(x1g=x1g, x1t=x1t, x1r2=x1r2, x2t=x2t)
                    nc.gpsimd.memset(x1g[64:128, :], -1.0)

                    # ---- phase 1: stage-1 convs -> x1 (feature-major) ----
                    xec = iop.tile([39, TNODES], f32, tag="xec")
                    fwc = xec[0:7, :]
                    bwc = xec[32:39, :]
                    nc.sync.dma_start(out=fwc, in_=xefw[:, c0:c0 + TNODES])
                    nc.scalar.dma_start(out=bwc, in_=xebw[:, c0:c0 + TNODES])
                    for ch in range(0 if "nos1" in ABLS else TNODES // 1024):
                        dl = slice(ch * 1024, (ch + 1) * 1024)
                        pf = ps_mix.tile([64, 1024], f32, tag="mix")
                        for h in range(2):
                            nc.tensor.matmul(
                                out=pf[:, h * 512:(h + 1) * 512], lhsT=t_s1wf,
                                rhs=fwc[:, ch * 1024 + h * 512:ch * 1024 + (h + 1) * 512],
                                start=True, stop=True)
                        fwa = s1st.tile([64, 1024], f32, tag="fwa")
                        nc.scalar.activation(out=fwa[:], in_=pf[:], func=AF.Relu,
                                             bias=t_s1b[:], scale=t_s1g[:])
                        pb = ps_mix.tile([64, 1024], f32, tag="mix")
                        for h in range(2):
                            nc.tensor.matmul(
                                out=pb[:, h * 512:(h + 1) * 512], lhsT=t_s1wb,
                                rhs=bwc[:, ch * 1024 + h * 512:ch * 1024 + (h + 1) * 512],
                                start=True, stop=True)
                        nc.scalar.activation(out=pb[:], in_=pb[:], func=AF.Relu,
                                             bias=t_s1b[:], scale=t_s1g[:])
                        nc.vector.tensor_tensor(out=x1g[0:64, dl], in0=fwa[:],
                                                in1=pb[:], op=OP.add)

                def ph2(t):
                    x1g, x1t, x1r2 = ST[t]["x1g"], ST[t]["x1t"], ST[t]["x1r2"]
                    nc.scalar.copy(out=x1t[:], in_=x1g[0:65, :])
                    nc.scalar.activation(out=x1r2[0:64, :], in_=x1g[0:64, :],
                                         func=AF.Copy, scale=2.0)
                    sq64 = s1st.tile([64, TNODES], f32, tag="sq64")
                    nc.scalar.activation(out=sq64[:], in_=x1g[0:64, :],
                                         func=AF.Square)
                    nc.sync.dma_start(out=x1r2[64:128, :], in_=sq64[:])

                def ph3(t):
                    x1g, x1r2 = ST[t]["x1g"], ST[t]["x1r2"]
                    SKIP3 = "noknn" in ABLS
                    d_loc = knnp.tile([128, NBLK * 16], f32, tag="dloc", name=f"d_loc{t}")
                    idxf = knnp.tile([128, NBLK * 8], u32, tag="idxf", name=f"idxf{t}")
                    m8f = knnp.tile([128, NBLK * 8], f32, tag="m8f", name=f"m8f{t}")
                    offf = knnp.tile([128, NBLK * 8], u32, tag="offf", name=f"offf{t}")
                    ST[t]["offf"] = offf
                    for r in range(0 if SKIP3 else NBLK // 8):
                        pd = ps_big.tile([128, 1024], f32, tag="big")
                        for n in range(8):
                            nt = r * 8 + n
                            sl = slice(nt * 128, (nt + 1) * 128)
                            nc.tensor.matmul(out=pd[:, n * 128:(n + 1) * 128],
                                             lhsT=x1g[:, sl], rhs=x1r2[:, sl],
                                             start=True, stop=True)
                        # extract local 16x16 diag blocks: negD [128p, n, 16q]
                        dfull = bstp.tile([128, 1024], f32, tag="dfull", bufs=1)
                        nc.scalar.copy(out=dfull[:], in_=pd[:])
                        for g in range(8):
                            src = dfull[16 * g:16 * g + 16, :].rearrange(
                                "p (n q) -> p n q", n=8)[:, :, 16 * g:16 * g + 16]
                            dst = d_loc[16 * g:16 * g + 16,
                                        r * 128:(r + 1) * 128].rearrange(
                                "p (n q) -> p n q", n=8)
                            eng = nc.sync if g % 2 == 0 else nc.scalar
                            eng.dma_start(out=dst, in_=src)
                    # poison pad column q=15
                    if SKIP3:
                        nc.vector.memset(offf[:], 0)
                    else:
                        nc.vector.memset(
                        d_loc[:].rearrange("p (n q) -> p n q", q=16)[:, :, 15:16],
                            BIG_NEG)
                    for nt in range(0 if SKIP3 else NBLK):
                        nc.vector.max(out=m8f[:, nt * 8:(nt + 1) * 8],
                                      in_=d_loc[:, nt * 16:(nt + 1) * 16])
                        nc.vector.max_index(out=idxf[:, nt * 8:(nt + 1) * 8],
                                            in_max=m8f[:, nt * 8:(nt + 1) * 8],
                                            in_values=d_loc[:, nt * 16:(nt + 1) * 16])
                    bslice = t_basep[:, 0:NBLK]
                    if not SKIP3:
                        nc.vector.tensor_tensor(
                        out=offf[:].rearrange("p (n k) -> p n k", k=8),
                        in0=idxf[:].rearrange("p (n k) -> p n k", k=8),
                            in1=bslice.unsqueeze(2).to_broadcast([128, NBLK, 8]),
                            op=OP.add)

                def ph4(t):
                    x1g = ST[t]["x1g"]
                    for r in range(0 if "nob" in ABLS else NBLK // 8):
                        pb8 = ps_big.tile([128, 1024], f32, tag="big")
                        for n in range(8):
                            nt = r * 8 + n
                            sl = slice(nt * 128, (nt + 1) * 128)
                            nc.tensor.matmul(out=pb8[:, n * 128:(n + 1) * 128],
                                             lhsT=x1g[0:64, sl], rhs=t_wdt[:],
                                             start=True, stop=True)
                        bstage = bstp.tile([128, 1024], f32, tag="bstage")
                        nc.scalar.copy(out=bstage[:], in_=pb8[:])
                        row0 = (r * 8) * 128
                        nc.sync.dma_start(
                            out=b_hbms[t][row0:row0 + 1024, :].rearrange(
                                "(n p) c -> p n c", p=128),
                            in_=bstage[:].rearrange("p (n c) -> p n c", n=8))

                def ph5(t):
                    x1t, x2t, offf = ST[t]["x1t"], ST[t]["x2t"], ST[t]["offf"]
                    for gi in range(0 if "nox2" in ABLS else NBLK // 4):
                        gath = gatp.tile([128, 4 * 5 * 128], f32, tag="gath")
                        if "nogather" not in ABLS:
                            for n in range(4):
                                nt = gi * 4 + n
                                for k in range(K):
                                    nc.gpsimd.indirect_dma_start(
                                        out=gath[:, (n * 5 + k) * 128:(n * 5 + k + 1) * 128],
                                        out_offset=None,
                                        in_=b_hbms[t][:],
                                        in_offset=bass.IndirectOffsetOnAxis(
                                            ap=offf[:, nt * 8 + k:nt * 8 + k + 1],
                                            axis=0))
                        else:
                            nc.vector.memset(gath[:], 0.0)
                        gv = gath[:].rearrange("p (n k c) -> p n k c", n=4, k=5)
                        mloc = knnp.tile([128, 4 * 128], f32, tag="mloc")
                        mv = mloc[:].rearrange("p (n c) -> p n c", n=4)
                        nc.vector.tensor_tensor(out=mv, in0=gv[:, :, 0, :],
                                                in1=gv[:, :, 1, :], op=OP.max)
                        for k in range(2, K):
                            nc.vector.tensor_tensor(out=mv, in0=mv,
                                                    in1=gv[:, :, k, :], op=OP.max)
                        pmt = ps_big.tile([128, 512], f32, tag="big")
                        gl = slice(gi * CHUNK, (gi + 1) * CHUNK)
                        for n in range(4):
                            nc.tensor.transpose(
                                out=pmt[:, n * 128:(n + 1) * 128],
                                in_=mloc[:, n * 128:(n + 1) * 128],
                                identity=ident[:])
                        pa = ps_mix.tile([128, CHUNK], f32, tag="mix")
                        nc.tensor.matmul(out=pa[:],
                                         lhsT=t_wa[:],
                                         rhs=x1t[:, gl],
                                         start=True, stop=True)
                        nc.scalar.copy(out=x2t[:, gl], in_=pa[:])
                        nc.vector.tensor_tensor(out=x2t[:, gl], in0=x2t[:, gl],
                                                in1=pmt[:], op=OP.add)
                        nc.scalar.activation(out=x2t[:, gl], in_=x2t[:, gl],
                                             func=AF.Relu)

                def ph6(t):
                    x1t, x2t = ST[t]["x1t"], ST[t]["x2t"]
                    for m in range(0 if "nol1" in ABLS else 8):
                        for cc in range(TNODES // 1024):
                            pl1 = ps_big.tile([128, 1024], f32, tag="big")
                            for h in range(2):
                                sl = slice(cc * 1024 + h * 512,
                                           cc * 1024 + (h + 1) * 512)
                                osl = slice(h * 512, (h + 1) * 512)
                                nc.tensor.matmul(
                                    out=pl1[:, osl],
                                    lhsT=t_wl1x1[:, m * 128:(m + 1) * 128],
                                    rhs=x1t[:, sl],
                                    start=True, stop=False)
                                nc.tensor.matmul(
                                    out=pl1[:, osl],
                                    lhsT=t_wl1x2[:, m * 128:(m + 1) * 128],
                                    rhs=x2t[:, sl],
                                    start=False, stop=True)
                            pv = pl1[:].rearrange("p (s q) -> p s q", q=16)[:, :, 0:15]
                            psl = slice(t * TSTRL + cc * 64,
                                        t * TSTRL + (cc + 1) * 64)
                            nc.vector.tensor_reduce(out=pooled[m][:, psl], in_=pv,
                                                    axis=AX.X, op=OP.max)

                def whole_body():
                    for base in range(0, NTILES, 2):
                        for ph in (ph1, ph2, ph3, ph4, ph5, ph6):
                            ph(base)
                            ph(base + 1)
                        ST.pop(base); ST.pop(base + 1)

                def head_body():
                    # ---- head: relu, m1, m2, m3 ----
                    h1 = pooled
                    for m in range(8):
                        nc.scalar.activation(out=h1[m][:], in_=h1[m][:], func=AF.Relu)
                    t1 = [headp.tile([128, BC], f32r, name=f"t1_{o}", tag=f"t1_{o}")
                          for o in range(4)]
                    wm1v = t_wm1[:].rearrange("p (a m) -> p a m", a=8)
                    for o in range(4):
                        pm1 = ps_big.tile([128, 1024], f32, tag="big")
                        for h in range(2):
                            osl = slice(h * 512, (h + 1) * 512)
                            for kc in range(8):
                                nc.tensor.matmul(
                                    out=pm1[:, osl],
                                    lhsT=wm1v[:, kc, o * 128:(o + 1) * 128],
                                    rhs=h1[kc][:, osl],
                                    start=(kc == 0), stop=False)
                            nc.tensor.matmul(
                                out=pm1[:, osl],
                                lhsT=t_bm1[:, o * 128:(o + 1) * 128],
                                rhs=ones_row[:, osl],
                                start=False, stop=True)
                        nc.scalar.activation(out=t1[o][:], in_=pm1[:], func=AF.Relu)
                    t2 = [headp.tile([128, BC], f32r, name=f"t2_{o}", tag=f"t2_{o}")
                          for o in range(2)]
                    wm2v = t_wm2[:].rearrange("p (a m) -> p a m", a=4)
                    for o in range(2):
                        pm2 = ps_big.tile([128, 1024], f32, tag="big")
                        for h in range(2):
                            osl = slice(h * 512, (h + 1) * 512)
                            for kc in range(4):
                                nc.tensor.matmul(
                                    out=pm2[:, osl],
                                    lhsT=wm2v[:, kc, o * 128:(o + 1) * 128],
                                    rhs=t1[kc][:, osl],
                                    start=(kc == 0), stop=False)
                            nc.tensor.matmul(
                                out=pm2[:, osl],
                                lhsT=t_bm2[:, o * 128:(o + 1) * 128],
                                rhs=ones_row[:, osl],
                                start=False, stop=True)
                        nc.scalar.activation(out=t2[o][:], in_=pm2[:], func=AF.Relu)
                    outs = headp.tile([2, BC], f32, tag="outs")
                    wm3v = t_wm3[:].rearrange("p (a m) -> p a m", a=2)
                    for h in range(2):
                        osl = slice(h * 512, (h + 1) * 512)
                        pm3 = ps_mix.tile([2, 512], f32, tag="mix")
                        for kc in range(2):
                            nc.tensor.matmul(
                                out=pm3[:],
                                lhsT=wm3v[:, kc, :],
                                rhs=t2[kc][:, osl],
                                start=(kc == 0), stop=False)
                        nc.tensor.matmul(out=pm3[:],
                                         lhsT=t_bm3[:],
                                         rhs=ones_row[:, osl],
                                         start=False, stop=True)
                        nc.scalar.copy(out=outs[:, osl], in_=pm3[:])
                    nc.sync.dma_start(out=out_t[:], in_=outs[:])

                REPEAT = int(os.environ.get("KREPEAT", "1"))
                if REPEAT > 1:
                    with tc.For_i(0, REPEAT, 1):
                        whole_body()
                        head_body()
                else:
                    whole_body()
                    head_body()

    nc.finalize()
    return nc


# ---------------- host-side prep ----------------
def _prep_inputs(pos, edge_index,
                 W_c1fw, b_c1fw, W_c1bw, b_c1bw, g_bn1, be_bn1,
                 W_e, b_e, g_e, be_e,
                 W_l1, b_l1, g_l1, be_l1,
                 W_m1, b_m1, g_m1, be_m1,
                 W_m2, b_m2, g_m2, be_m2,
                 W_m3, b_m3):
    f = np.float32
    pos = np.asarray(pos, f)
    E = edge_index.shape[1]
    N = E // 2
    second = np.asarray(edge_index[:, N:])
    first = second[:, ::-1]
    src = np.concatenate([first[0], second[0]])
    dst = np.concatenate([first[1], second[1]])
    xe = np.concatenate([pos[dst] - pos[src], pos[src]], axis=1).astype(f)
    xe = xe.reshape(2 * B_FULL, P, 2 * D)
    fw = xe[:B_FULL]
    bw = xe[B_FULL:][::-1, ::-1, :]

    def pad_t(a):
        # [B, 15, 6] -> per-core feature-major [7, NODES] with ones row
        out = np.zeros((B_FULL, PP, 7), f)
        out[:, :P, :6] = a
        out[:, :, 6] = 1.0
        out = out.reshape(NCORES, NODES, 7)
        return np.ascontiguousarray(out.transpose(0, 2, 1))

    xefw = pad_t(fw)
    xebw = pad_t(bw)

    sq = np.sqrt(np.asarray(1.0 + EPS, f))
    g1 = (np.asarray(g_bn1, f) / sq)[:, None]
    be1 = np.asarray(be_bn1, f)[:, None]
    s1wf = np.ascontiguousarray(
        np.concatenate([np.asarray(W_c1fw, f), np.asarray(b_c1fw, f)[:, None]], 1).T)
    s1wb = np.ascontiguousarray(
        np.concatenate([np.asarray(W_c1bw, f), np.asarray(b_c1bw, f)[:, None]], 1).T)

    W_e = np.asarray(W_e, f)
    Wi, Wd = W_e[:, :64], W_e[:, 64:]
    wa = np.ascontiguousarray(
        np.concatenate([(Wi - Wd).T, -np.asarray(b_e, f)[None, :]], 0))
    wdt = np.ascontiguousarray(Wd.T)

    ge = np.asarray(g_e, f) / sq
    bee = np.asarray(be_e, f)
    W_l1 = np.asarray(W_l1, f)
    Wl1x1 = W_l1[:, :64]
    Wl1x2 = W_l1[:, 64:] * ge[None, :]
    bl1 = np.asarray(b_l1, f) + W_l1[:, 64:] @ bee
    wl1x1 = np.ascontiguousarray(np.concatenate([Wl1x1.T, -bl1[None, :]], 0))
    wl1x2 = np.ascontiguousarray(Wl1x2.T)

    def m_fold(W, b, g_prev, be_prev, kchunks):
        # fold previous-layer bn affine into this layer; arrange lhsT
        # [K, M] -> [128, kchunks*M]
        W = np.asarray(W, f)
        gp = np.asarray(g_prev, f) / sq
        Wf = W * gp[None, :]
        bf = np.asarray(b, f) + W @ np.asarray(be_prev, f)
        lhsT = Wf.T  # [K, M]
        Kd, Md = lhsT.shape
        arr = lhsT.reshape(kchunks, 128, Md).transpose(1, 0, 2).reshape(128, -1)
        return np.ascontiguousarray(arr), bf[None, :]

    wm1a, bm1v = m_fold(W_m1, b_m1, g_l1, be_l1, 8)
    wm2a, bm2v = m_fold(W_m2, b_m2, g_m1, be_m1, 4)
    wm3a, bm3v = m_fold(W_m3, b_m3, g_m2, be_m2, 2)

    pidx = np.arange(128, dtype=np.uint32)
    nidx = np.arange(NODES // NTILES // 128, dtype=np.uint32)
    basep = (nidx[None, :] * 128 + (pidx[:, None] // 16) * 16).astype(np.uint32)

    shared = {
        "s1wf": s1wf, "s1wb": s1wb, "s1g": g1, "s1b": be1,
        "wa": wa, "wdt": wdt,
        "wl1x1": wl1x1, "wl1x2": wl1x2,
        "wm1": wm1a, "bm1": bm1v,
        "wm2": wm2a, "bm2": bm2v,
        "wm3": wm3a, "bm3": bm3v,
        "basep": basep,
        "onesr": np.ones((1, BC), f),
    }
    in_maps = []
    for c in range(NCORES):
        m = dict(shared)
        m["xefw"] = xefw[c]
        m["xebw"] = xebw[c]
        in_maps.append(m)
    return in_maps


def _get_runner():
    if "runner" in _CACHE:
        return _CACHE["runner"]
    from concourse.bass_utils import run_bass_kernel_spmd
    nc = _build_program()
    _CACHE["nc"] = nc

    def runner(in_maps):
        return run_bass_kernel_spmd(nc, in_maps, list(range(NCORES))).results

    _CACHE["runner"] = runner
    return runner


def kernel(**inputs):
    in_maps = _prep_inputs(**inputs)
    results = _get_runner()(in_maps)
    out = np.empty((B_FULL, NCLS), np.float32)
    for c in range(NCORES):
        out[c * BC:(c + 1) * BC, :] = results[c]["out"].T
    return out

